# revision 1
# baseline (speedup 1.0000x reference)
"""Trainium2 Bass kernel for nn_EntityMapping (P=16 independent MLPs over a
shared entity batch).

Sharding: the 16 partition-MLPs are split across 8 NeuronCores (2 per core,
expert-parallel); the embedding batch is replicated. Activations are kept
feature-major [feature, batch] on-chip so every layer is a chain of
128x128-stationary matmuls with the batch streaming through the PE array.
Matmuls run in float32r (full-rate fp32 on TRN2's PE at N>=256; inputs are
rounded to fp32r by DVE producer ops as walrus requires).

Weights are host-packed in PE consumption order (p,j,k) and streamed in
eighths, DMA-issued and DVE-rounded in exactly the order the PE consumes
them, so the first matmul fires ~12us in and never starves; 10 junk
matmuls on a memset tile warm the PE clock (HAM) during the load window.

Measured: ~269us HW exec per core (all 8 cores within 1.2us), fro rel err
1.34e-5 vs the fp32 reference. Steady state runs 233ns per 512-col matmul
(fp32r streaming floor 213ns + Tile per-instruction semaphore tick) with
zero scheduling stalls; head/tail are the fixed Tile init (7.2us) and
drain barrier (9.5us); the ramp is HBM-wire-bound.
"""

import os
import sys

import numpy as np

if "jax" not in sys.modules and os.environ.get("JAX_PLATFORMS") == "cpu":
    # don't let a cpu pin hide the axon/neuron backend the kernel runs on
    os.environ["JAX_PLATFORMS"] = ""

try:
    import concourse.bass as bass  # noqa: F401
except ImportError:  # harness runs kernel.py from a bare directory
    sys.path.insert(0, "/opt/trn_rl_repo")

import concourse.mybir as mybir
import concourse.tile as tile
from concourse import bacc
from concourse.bass_utils import run_bass_kernel_spmd

F32 = mybir.dt.float32
F32R = mybir.dt.float32r
RELU = mybir.ActivationFunctionType.Relu
SIGMOID = mybir.ActivationFunctionType.Sigmoid
COPY = mybir.ActivationFunctionType.Copy

P_TOTAL = 16  # independent MLP partitions
E = 512  # entity/embedding dim
H = 512  # hidden dim
N = 8192  # batch (entities)
N_CORES = 8
P_PER = P_TOTAL // N_CORES  # 2 MLPs per core
KC = E // 128  # 4 contraction chunks per layer
JC = H // 128  # 4 output-feature chunks per layer
NCH = 512  # batch columns per n-chunk (= fp32 moving-operand max = PSUM bank)
NCHUNKS = N // NCH  # 16
NW = P_PER * KC * JC  # 32 weight tiles per layer
PIECE = JC * 128  # weight piece = one (p,j) group of KC tiles = 512 cols
WARMUP_MM = 10  # junk matmuls to warm the PE clock during weight load
# (sized so HAM's ~3.4us busy window fills, but the queue drains right as
# the first real matmul's inputs land — overshoot delays real work)


def _build():
    nc = bacc.Bacc(
        "TRN2", target_bir_lowering=False, debug=False, num_devices=N_CORES
    )
    # All inputs pre-packed on host into SBUF-layout [128, cols]:
    eT_dram = nc.dram_tensor("eT", [E, N], F32, kind="ExternalInput")
    w0_dram = nc.dram_tensor("w0", [128, NW * 128], F32, kind="ExternalInput")
    w1_dram = nc.dram_tensor("w1", [128, NW * 128], F32, kind="ExternalInput")
    b0_dram = nc.dram_tensor("b0", [128, P_PER * JC], F32, kind="ExternalInput")
    b1_dram = nc.dram_tensor("b1", [128, P_PER * JC], F32, kind="ExternalInput")
    w2_dram = nc.dram_tensor("w2", [128, P_PER * KC], F32, kind="ExternalInput")
    b2_dram = nc.dram_tensor("b2", [1, P_PER], F32, kind="ExternalInput")
    out_dram = nc.dram_tensor("out", [P_PER, N], F32, kind="ExternalOutput")

    # eT viewed as [ki, k, n] for per-k chunk DMAs
    eT_v = eT_dram.rearrange("(k ki) n -> ki k n", ki=128)

    with tile.TileContext(nc) as tc:
        with (
            tc.tile_pool(name="wconst", bufs=1) as wconst,
            tc.tile_pool(name="wstage", bufs=1) as wstage,
            tc.tile_pool(name="warm", bufs=1) as warm_pool,
            tc.tile_pool(name="et", bufs=3) as et_pool,
            tc.tile_pool(name="etr", bufs=3) as etr_pool,
            tc.tile_pool(name="act", bufs=2) as act_pool,
            tc.tile_pool(name="osb", bufs=4) as out_pool,
            tc.tile_pool(name="mmps", bufs=6, space="PSUM") as ps_mm,
            tc.tile_pool(name="l2ps", bufs=2, space="PSUM") as ps_l2,
        ):
            # persistent rounded weights + staging
            w0_r = wconst.tile([128, NW, 128], F32R, tag="w0r")
            w1_r = wconst.tile([128, NW, 128], F32R, tag="w1r")
            w0_rf = w0_r[:].rearrange("p a b -> p (a b)")
            w1_rf = w1_r[:].rearrange("p a b -> p (a b)")
            w0_st = wstage.tile([128, NW * 128], F32, tag="s0")
            w1_st = wstage.tile([128, NW * 128], F32, tag="s1")

            def w_dma(st, wd, q):  # stream piece q of a weight layer
                nc.sync.dma_start(
                    st[:, q * PIECE : (q + 1) * PIECE],
                    wd[:, q * PIECE : (q + 1) * PIECE],
                )

            def w_round(st, wrf, q, alt=False):  # f32r-round piece q
                if alt:
                    nc.scalar.activation(
                        wrf[:, q * PIECE : (q + 1) * PIECE],
                        st[:, q * PIECE : (q + 1) * PIECE],
                        COPY,
                    )
                else:
                    nc.vector.tensor_copy(
                        wrf[:, q * PIECE : (q + 1) * PIECE],
                        st[:, q * PIECE : (q + 1) * PIECE],
                    )

            # --- PE warmup: junk matmuls on a memset tile so HAM is at
            # K=8/8 when the first real matmul issues ---
            wm_f = warm_pool.tile([128, 640], F32, tag="wmf")
            nc.gpsimd.memset(wm_f[:], 0.0)
            wm_r = warm_pool.tile([128, 640], F32R, tag="wmr")
            nc.vector.tensor_copy(wm_r[:], wm_f[:])
            ps_warm = ps_l2.tile([128, 512], F32, tag="l2")
            for i in range(WARMUP_MM):
                nc.tensor.matmul(
                    ps_warm[:],
                    wm_r[:, 0:128],
                    wm_r[:, 128:640],
                    start=(i == 0),
                    stop=(i == WARMUP_MM - 1),
                )

            # first weight piece + small constants up front
            w_dma(w0_st, w0_dram, 0)
            b0_sb = wconst.tile([128, P_PER * JC], F32, tag="b0")
            nc.sync.dma_start(b0_sb[:], b0_dram[:])
            b1_sb = wconst.tile([128, P_PER * JC], F32, tag="b1")
            nc.sync.dma_start(b1_sb[:], b1_dram[:])
            b2_sb = wconst.tile([1, P_PER], F32, tag="b2")
            nc.sync.dma_start(b2_sb[:], b2_dram[:])
            w2_sb = wconst.tile([128, P_PER * KC], F32, tag="w2sb")
            nc.sync.dma_start(w2_sb[:], w2_dram[:])
            w_round(w0_st, w0_rf, 0)
            # ones column for the L2 partition-reduction matmul
            ones_f = warm_pool.tile([128, 1], F32, tag="onef")
            nc.gpsimd.memset(ones_f[:], 1.0)
            ones_r = warm_pool.tile([128, 1], F32R, tag="oner")
            nc.vector.tensor_copy(ones_r[:], ones_f[:])
            # f32r w2 for the final chunk's direct-matmul L2 (shorter tail)
            w2_r = wconst.tile([128, P_PER * KC], F32R, tag="w2r")
            nc.vector.tensor_copy(w2_r[:], w2_sb[:])

            def l0(p, et):
                h = act_pool.tile([128, JC, NCH], F32R, tag="h")
                for j in range(JC):
                    ps = ps_mm.tile([128, NCH], F32, tag="mm")
                    for k in range(KC):
                        wi = (p * JC + j) * KC + k
                        nc.tensor.matmul(
                            ps[:], w0_r[:, wi, :], et[:, k, :],
                            start=(k == 0), stop=(k == KC - 1),
                        )
                    nc.scalar.activation(
                        h[:, j, :], ps[:], RELU,
                        bias=b0_sb[:, p * JC + j : p * JC + j + 1],
                    )
                return h

            def l12(p, h, n0, direct=False):
                # L1 + L2 fused: after each relu j, scale by w2[j] on DVE and
                # accumulate the k-sum g incrementally, so the per-chunk tail
                # is just mul+add+ones-matmul+sigmoid.
                # u[n] = sum_feat w2[feat]*h2[feat,n] = ones^T g.
                # direct=True (final chunk) reduces via 4 w2-matmuls instead,
                # skipping the DVE chain for a shorter kernel tail.
                h2 = act_pool.tile([128, JC, NCH], F32R, tag="h2")
                g = act_pool.tile([128, NCH], F32R, tag="g")
                r = ps_l2.tile([1, NCH], F32, tag="l2")
                for j in range(JC):
                    ps = ps_mm.tile([128, NCH], F32, tag="mm")
                    for k in range(KC):
                        wi = (p * JC + j) * KC + k
                        nc.tensor.matmul(
                            ps[:], w1_r[:, wi, :], h[:, k, :],
                            start=(k == 0), stop=(k == KC - 1),
                        )
                    nc.scalar.activation(
                        h2[:, j, :], ps[:], RELU,
                        bias=b1_sb[:, p * JC + j : p * JC + j + 1],
                    )
                    if direct:
                        nc.tensor.matmul(
                            r[:], w2_r[:, p * KC + j : p * KC + j + 1],
                            h2[:, j, :], start=(j == 0), stop=(j == JC - 1),
                        )
                        continue
                    nc.vector.tensor_scalar_mul(
                        h2[:, j, :], h2[:, j, :],
                        w2_sb[:, p * KC + j : p * KC + j + 1],
                    )
                    if j == 1:
                        nc.vector.tensor_add(g[:], h2[:, 0, :], h2[:, 1, :])
                    elif j > 1:
                        nc.vector.tensor_add(g[:], g[:], h2[:, j, :])
                if not direct:
                    nc.tensor.matmul(r[:], ones_r[:], g[:], start=True, stop=True)
                o = out_pool.tile([1, NCH], F32, tag="o")
                nc.scalar.activation(o[:], r[:], SIGMOID, bias=b2_sb[0:1, p : p + 1])
                nc.sync.dma_start(out_dram[p : p + 1, n0 : n0 + NCH], o[:])

            def load_et(c, eng=None):
                # et0 rides gpsimd (issues immediately); later chunks ride
                # sync BEHIND the weight pieces so they can't steal wire
                # bandwidth from w1 during the ramp.  During the ramp (c<2)
                # the f32r casts alternate DVE/ACT so they finish sooner.
                n0 = c * NCH
                et_f = et_pool.tile([128, KC, NCH], F32, tag="et")
                eng = eng or nc.sync
                for k in range(KC):
                    eng.dma_start(et_f[:, k, :], eT_v[:, k, n0 : n0 + NCH])
                et = etr_pool.tile([128, KC, NCH], F32R, tag="etr")
                for k in range(KC):
                    if c < 2 and k % 2 == 1:
                        nc.scalar.activation(et[:, k, :], et_f[:, k, :], COPY)
                    else:
                        nc.vector.tensor_copy(et[:, k, :], et_f[:, k, :])
                return et

            # --- chunk 0: L0 for both partitions first (needs only w0+et0),
            # giving the wire time to deliver w1; loads interleaved in
            # consumption order ---
            et0 = load_et(0, eng=nc.gpsimd)
            for q in range(1, P_PER * JC):  # w0 pieces 1..7
                w_dma(w0_st, w0_dram, q)
            for q in range(P_PER * JC):  # all w1 pieces
                w_dma(w1_st, w1_dram, q)
            for q in range(1, P_PER * JC):
                w_round(w0_st, w0_rf, q, alt=(q % 2 == 1))
            h_0 = l0(0, et0)
            h_1 = l0(1, et0)
            et1 = load_et(1)
            for q in range(P_PER * JC):
                w_round(w1_st, w1_rf, q, alt=(q % 2 == 1))
            l12(0, h_0, 0)
            l12(1, h_1, 0)

            # --- steady-state loop ---
            ets = {1: et1}
            for c in range(1, NCHUNKS):
                et = ets.pop(c)
                if c + 1 < NCHUNKS:
                    ets[c + 1] = load_et(c + 1)
                n0 = c * NCH
                for p in range(P_PER):
                    h = l0(p, et)
                    last = c == NCHUNKS - 1 and p == P_PER - 1
                    l12(p, h, n0, direct=last)

    nc.compile()
    return nc


_NC_CACHE = None


def _get_nc():
    global _NC_CACHE
    if _NC_CACHE is None:
        _NC_CACHE = _build()
    return _NC_CACHE


def _make_in_maps(e_embedding, W0, b0, W1, b1, W2, b2):
    e = np.asarray(e_embedding, dtype=np.float32)
    W0 = np.asarray(W0, dtype=np.float32)
    b0 = np.asarray(b0, dtype=np.float32)
    W1 = np.asarray(W1, dtype=np.float32)
    b1 = np.asarray(b1, dtype=np.float32)
    W2 = np.asarray(W2, dtype=np.float32)
    b2 = np.asarray(b2, dtype=np.float32)

    eT = np.ascontiguousarray(e.T)  # [E, N]
    in_maps = []
    for cid in range(N_CORES):
        sl = slice(P_PER * cid, P_PER * (cid + 1))
        # SBUF layout [ki, ((p j k) ji)] — PE consumption order
        w0t = np.ascontiguousarray(
            W0[sl]
            .reshape(P_PER, KC, 128, JC, 128)
            .transpose(2, 0, 3, 1, 4)
            .reshape(128, NW * 128)
        )
        w1t = np.ascontiguousarray(
            W1[sl]
            .reshape(P_PER, KC, 128, JC, 128)
            .transpose(2, 0, 3, 1, 4)
            .reshape(128, NW * 128)
        )
        # [ki, (p j)] bias columns
        b0t = np.ascontiguousarray(
            b0[sl].reshape(P_PER, JC, 128).transpose(2, 0, 1).reshape(128, -1)
        )
        b1t = np.ascontiguousarray(
            b1[sl].reshape(P_PER, JC, 128).transpose(2, 0, 1).reshape(128, -1)
        )
        w2t = np.ascontiguousarray(
            W2[sl, :, 0].reshape(P_PER, KC, 128).transpose(2, 0, 1).reshape(128, -1)
        )
        b2t = np.ascontiguousarray(b2[sl].reshape(1, P_PER))
        in_maps.append(
            {"eT": eT, "w0": w0t, "b0": b0t, "w1": w1t, "b1": b1t,
             "w2": w2t, "b2": b2t}
        )
    return in_maps


def kernel_with_results(trace=False, **inputs):
    nc = _get_nc()
    in_maps = _make_in_maps(**inputs)
    try:
        res = run_bass_kernel_spmd(
            nc, in_maps, core_ids=list(range(N_CORES)), trace=trace
        )
    except Exception:
        # the first PJRT compile in a fresh container can fail transiently;
        # one retry reuses the primed NEFF cache
        res = run_bass_kernel_spmd(
            nc, in_maps, core_ids=list(range(N_CORES)), trace=trace
        )
    full = np.concatenate([r["out"] for r in res.results], axis=0)  # [16, N]
    out = np.ascontiguousarray(full.T).astype(np.float32)  # [N, 16]
    return out, res


def kernel(**inputs):
    out, _ = kernel_with_results(trace=False, **inputs)
    return out



# revision 13
# speedup vs baseline: 1.6216x; 1.6216x over previous
"""Trainium2 Bass kernel for nn_EntityMapping (P=16 independent MLPs over a
shared entity batch).

Sharding: the 16 partition-MLPs are split across 8 NeuronCores (2 per core,
expert-parallel); the embedding batch is replicated. Activations are kept
feature-major [feature, batch] on-chip so every layer is a chain of
128x128-stationary matmuls with the batch streaming through the PE array.

All matmuls run in fp8 (e4m3) DoubleRow perf mode: one instruction contracts
over 256 rows (two 128-k-chunks) at 0.5 PE cycles per output column — 4x the
per-k-chunk throughput of fp32r. Inputs/weights are quantized host-side with
power-of-2 scales chosen so each layer's PSUM lands directly in the next
layer's fp8 units (SE*SW0 = SH1, SH1*SW1 = SH2): the relu is then a pure
(x + bias, max 0) with no rescale — a single activation on ACT or a single
dual-op tensor_scalar on DVE. GPSIMD cannot read PSUM, so relu work is split
ACT/DVE only; to amortize the per-instruction PSUM-access overhead, batch
chunks are processed in PAIRS: the same j-tile of two adjacent 512-column
chunks lands in adjacent PSUM banks and one [128, 2x512] relu (the bias is
per-feature, so it is shared) covers both. The final w2-dot runs as 2
DoubleRow matmuls per MLP/chunk into PSUM partitions 0/32 and one strided
sigmoid [2, 1024] covers both MLPs and both chunks.

Quantization error (validated on host vs the fp32 reference): rel_fro
~3.2e-3, max |pre-fp8 activation| ~104 of the 240 e4m3 range.
"""

import os
import sys

import numpy as np

if "jax" not in sys.modules and os.environ.get("JAX_PLATFORMS") == "cpu":
    # don't let a cpu pin hide the axon/neuron backend the kernel runs on
    os.environ["JAX_PLATFORMS"] = ""

try:
    import concourse.bass as bass  # noqa: F401
except ImportError:  # harness runs kernel.py from a bare directory
    sys.path.insert(0, "/opt/trn_rl_repo")

import ml_dtypes
import concourse.mybir as mybir
import concourse.tile as tile
from concourse import bacc
from concourse.bass_utils import run_bass_kernel_spmd

F32 = mybir.dt.float32
F8 = mybir.dt.float8e4
RELU = mybir.ActivationFunctionType.Relu
SIGMOID = mybir.ActivationFunctionType.Sigmoid
DR = mybir.MatmulPerfMode.DoubleRow
ADD = mybir.AluOpType.add
MAX = mybir.AluOpType.max

F8NP = ml_dtypes.float8_e4m3  # TRN-style e4m3 (max +-240), not OCP e4m3fn

P_TOTAL = 16  # independent MLP partitions
E = 512  # entity/embedding dim
H = 512  # hidden dim
N = 8192  # batch (entities)
N_CORES = 8
P_PER = P_TOTAL // N_CORES  # 2 MLPs per core
JC = 4  # 128-wide output-feature tiles per layer
KP = 2  # DoubleRow pairs per 512-deep contraction (2 x 256)
NCH = 512  # batch columns per chunk (= PSUM bank of fp32)
NDC = N // (2 * NCH)  # 8 double-chunks
NW = P_PER * JC * KP  # 16 DoubleRow weight tiles [128,2,128] per layer

# power-of-2 quantization scales; SE*SW0 = SH1 and SH1*SW1 = SH2 make the
# PSUM arrive already in the next layer's fp8 units (relu needs no rescale)
SE, SW0, SW1, SW2 = 2.0, 16.0, 2.0, 128.0
SH1 = SE * SW0  # 32
SH2 = SH1 * SW1  # 64
SC2 = 1.0 / (SH2 * SW2)  # 1/8192, applied inside the sigmoid activation

WARMUP_MM = 10  # junk matmuls to warm the PE clock during the load window


def _build():
    nc = bacc.Bacc(
        "TRN2", target_bir_lowering=False, debug=False, num_devices=N_CORES
    )
    # All inputs pre-quantized and pre-packed on host into SBUF layout:
    eT_dram = nc.dram_tensor("eT", [128, KP * 2 * N], F8, kind="ExternalInput")
    w0_dram = nc.dram_tensor("w0", [128, NW * 2 * 128], F8, kind="ExternalInput")
    w1_dram = nc.dram_tensor("w1", [128, NW * 2 * 128], F8, kind="ExternalInput")
    # dual-fp8 Ldweights needs a stationary free dim >= 32 and DoubleRow
    # matmuls need dst partition 0: w2 is padded to 64 zero columns with
    # MLP p's vector at column 32*p, so one accumulated PSUM region holds
    # p0's dot at partition 0 and p1's at partition 32
    w2_dram = nc.dram_tensor(
        "w2", [128, P_PER * KP * 2 * 64], F8, kind="ExternalInput"
    )
    b0_dram = nc.dram_tensor("b0", [128, P_PER * JC], F32, kind="ExternalInput")
    b1_dram = nc.dram_tensor("b1", [128, P_PER * JC], F32, kind="ExternalInput")
    b2_dram = nc.dram_tensor("b2", [P_PER, 1], F32, kind="ExternalInput")
    out_dram = nc.dram_tensor("out", [P_PER, N], F32, kind="ExternalOutput")

    eT_v = eT_dram.rearrange("p (kp two n) -> p kp two n", kp=KP, two=2)

    with tile.TileContext(nc) as tc:
        with (
            tc.tile_pool(name="wconst", bufs=1) as wconst,
            tc.tile_pool(name="warm", bufs=1) as warm_pool,
            tc.tile_pool(name="et", bufs=1) as et_pool,
            tc.tile_pool(name="h1", bufs=3) as h1_pool,
            tc.tile_pool(name="h2", bufs=3) as h2_pool,
            tc.tile_pool(name="osb", bufs=3) as out_pool,
            tc.tile_pool(name="mmps", bufs=3, space="PSUM") as ps_mm,
            tc.tile_pool(name="l2ps", bufs=1, space="PSUM") as ps_l2,
        ):
            # --- PE warmup: junk DoubleRow matmuls on a memset tile so the
            # PE clock is ramped when the first real matmul issues ---
            wm = warm_pool.tile([128, 2, 640], F8, tag="wm")
            nc.gpsimd.memset(wm[:], 0.0)
            ps_warm = ps_mm.tile([128, 2, NCH], F32, tag="mm")
            for i in range(WARMUP_MM):
                nc.tensor.matmul(
                    ps_warm[:, 0, :],
                    wm[:, :, 0:128],
                    wm[:, :, 128:640],
                    start=(i == 0),
                    stop=(i == WARMUP_MM - 1),
                    perf_mode=DR,
                )

            # --- whole eT resident in SBUF (32KB/partition in fp8); issued
            # on the ACT HW-DGE queue (ACT is idle this early), column-split
            # so chunk 0 unblocks immediately ---
            et = et_pool.tile([128, KP, 2, N], F8, tag="et")
            for lo, hi in ((0, 1024), (1024, 4608), (4608, N)):
                for q in range(KP):
                    for t in range(2):
                        nc.scalar.dma_start(
                            et[:, q, t, lo:hi], eT_v[:, q, t, lo:hi]
                        )

            # --- weights + consts on the SP HW-DGE queue, in consumption
            # order: w0 pieces (p,j), small consts, w1 pieces ---
            w0s = wconst.tile([128, NW, 2, 128], F8, tag="w0")
            w1s = wconst.tile([128, NW, 2, 128], F8, tag="w1")
            w2s = wconst.tile([128, P_PER, KP, 2, 64], F8, tag="w2")
            b0s = wconst.tile([128, P_PER * JC], F32, tag="b0")
            b1s = wconst.tile([128, P_PER * JC], F32, tag="b1")
            b2s = wconst.tile([33, 1], F32, tag="b2")

            w0sv = w0s[:].rearrange("p i two m -> p (i two m)")
            w1sv = w1s[:].rearrange("p i two m -> p (i two m)")
            PIECE = KP * 2 * 128  # one (p,j) group = 512 fp8 cols

            def w_dma(dst_flat, src, p, j):
                q0 = (p * JC + j) * PIECE
                nc.sync.dma_start(
                    dst_flat[:, q0 : q0 + PIECE],
                    src[:, q0 : q0 + PIECE],
                )

            for p in range(P_PER):
                for j in range(JC):
                    w_dma(w0sv, w0_dram, p, j)
            nc.sync.dma_start(b0s[:], b0_dram[:])
            nc.sync.dma_start(b1s[:], b1_dram[:])
            nc.sync.dma_start(
                w2s[:].rearrange("p a q two m -> p (a q two m)"), w2_dram[:]
            )
            nc.sync.dma_start(b2s[0:1, :], b2_dram[0:1, :])
            nc.sync.dma_start(b2s[32:33, :], b2_dram[1:2, :])
            for p in range(P_PER):
                for j in range(JC):
                    w_dma(w1sv, w1_dram, p, j)

            # --- relu dispatch: alternate ACT/DVE (ACT also owns the
            # per-double-chunk sigmoid, DVE is slightly slower per op) ---
            relu_ctr = [0]

            def relu(dst, ps, bias):
                relu_ctr[0] += 1
                if relu_ctr[0] % 2:
                    nc.vector.tensor_scalar(dst, ps[:], bias, 0.0, ADD, MAX)
                else:
                    nc.scalar.activation(dst, ps[:], RELU, bias=bias)

            # h tiles hold one double-chunk: [feature128, j, cc, col]
            def l0(p, dc):
                n0 = dc * 2 * NCH
                h1 = h1_pool.tile([128, JC, 2, NCH], F8, tag="h1")
                for j in range(JC):
                    ps = ps_mm.tile([128, 2, NCH], F32, tag="mm")
                    for cc in range(2):
                        nn = n0 + cc * NCH
                        for q in range(KP):
                            wi = (p * JC + j) * KP + q
                            nc.tensor.matmul(
                                ps[:, cc, :],
                                w0s[:, wi, :, :],
                                et[:, q, :, nn : nn + NCH],
                                start=(q == 0),
                                stop=(q == KP - 1),
                                perf_mode=DR,
                            )
                    col = p * JC + j
                    relu(h1[:, j, :, :], ps, b0s[:, col : col + 1])
                return h1

            def l1(p, h1):
                h2 = h2_pool.tile([128, JC, 2, NCH], F8, tag="h2")
                for j in range(JC):
                    ps = ps_mm.tile([128, 2, NCH], F32, tag="mm")
                    for cc in range(2):
                        for q in range(KP):
                            wi = (p * JC + j) * KP + q
                            nc.tensor.matmul(
                                ps[:, cc, :],
                                w1s[:, wi, :, :],
                                h1[:, 2 * q : 2 * q + 2, cc, :],
                                start=(q == 0),
                                stop=(q == KP - 1),
                                perf_mode=DR,
                            )
                    col = p * JC + j
                    relu(h2[:, j, :, :], ps, b1s[:, col : col + 1])
                return h2

            def l2(dc, h2_by_p):
                n0 = dc * 2 * NCH
                r = ps_l2.tile([64, 2, NCH], F32, tag="l2")
                for cc in range(2):
                    for p, h2 in enumerate(h2_by_p):
                        for q in range(KP):
                            nc.tensor.matmul(
                                r[:, cc, :],
                                w2s[:, p, q, :, :],
                                h2[:, 2 * q : 2 * q + 2, cc, :],
                                start=(p == 0 and q == 0),
                                stop=(p == P_PER - 1 and q == KP - 1),
                                perf_mode=DR,
                            )
                o = out_pool.tile([33, 2, NCH], F32, tag="o")
                nc.scalar.activation(
                    o[0:1, :, :], r[0:1, :, :], SIGMOID,
                    bias=b2s[0:1, :], scale=SC2,
                )
                nc.scalar.activation(
                    o[32:33, :, :], r[32:33, :, :], SIGMOID,
                    bias=b2s[32:33, :], scale=SC2,
                )
                nc.sync.dma_start(out_dram[0:1, n0 : n0 + 2 * NCH], o[0:1, :, :])
                nc.sync.dma_start(out_dram[1:2, n0 : n0 + 2 * NCH], o[32:33, :, :])

            # --- software-pipelined main loop over double-chunks: dc+1's L0
            # runs between dc's L1 and L2, giving every relu a full PE-block
            # of slack before its consumer ---
            h1s = [l0(0, 0), l0(1, 0)]
            for dc in range(NDC):
                h2s = [l1(0, h1s[0]), l1(1, h1s[1])]
                if dc + 1 < NDC:
                    h1s = [l0(0, dc + 1), l0(1, dc + 1)]
                l2(dc, h2s)

    nc.compile()
    return nc


_NC_CACHE = None


def _get_nc():
    global _NC_CACHE
    if _NC_CACHE is None:
        _NC_CACHE = _build()
    return _NC_CACHE


def _q8(x, scale):
    return (np.asarray(x, dtype=np.float32) * scale).astype(F8NP)


def _make_in_maps(e_embedding, W0, b0, W1, b1, W2, b2):
    e = np.asarray(e_embedding, dtype=np.float32)
    W0 = np.asarray(W0, dtype=np.float32)
    b0 = np.asarray(b0, dtype=np.float32)
    W1 = np.asarray(W1, dtype=np.float32)
    b1 = np.asarray(b1, dtype=np.float32)
    W2 = np.asarray(W2, dtype=np.float32)
    b2 = np.asarray(b2, dtype=np.float32)

    # eT [E, N] -> [ki, kp, two, n] fp8 (replicated to all cores)
    eTq = np.ascontiguousarray(
        _q8(e.T, SE).reshape(KP, 2, 128, N).transpose(2, 0, 1, 3).reshape(128, -1)
    )

    def wpack(W, s):  # [pp, 512, 512] -> [ki, (p j q two m)] fp8
        return np.ascontiguousarray(
            _q8(W, s)
            .reshape(P_PER, KP, 2, 128, JC, 128)
            .transpose(3, 0, 4, 1, 2, 5)
            .reshape(128, -1)
        )

    in_maps = []
    for cid in range(N_CORES):
        sl = slice(P_PER * cid, P_PER * (cid + 1))
        w0t = wpack(W0[sl], SW0)
        w1t = wpack(W1[sl], SW1)
        w2q = (
            _q8(W2[sl, :, 0], SW2)
            .reshape(P_PER, KP, 2, 128)
            .transpose(3, 0, 1, 2)
        )  # [128, p, q, two]
        w2t = np.zeros((128, P_PER, KP, 2, 64), dtype=F8NP)
        for p in range(P_PER):
            w2t[:, p, :, :, 32 * p] = w2q[:, p]
        w2t = np.ascontiguousarray(w2t.reshape(128, -1))
        b0t = np.ascontiguousarray(
            (b0[sl] * SH1).reshape(P_PER, JC, 128).transpose(2, 0, 1).reshape(128, -1)
        )
        b1t = np.ascontiguousarray(
            (b1[sl] * SH2).reshape(P_PER, JC, 128).transpose(2, 0, 1).reshape(128, -1)
        )
        b2t = np.ascontiguousarray(b2[sl].reshape(P_PER, 1))
        in_maps.append(
            {"eT": eTq, "w0": w0t, "b0": b0t, "w1": w1t, "b1": b1t,
             "w2": w2t, "b2": b2t}
        )
    return in_maps


def kernel_with_results(trace=False, **inputs):
    nc = _get_nc()
    in_maps = _make_in_maps(**inputs)
    try:
        res = run_bass_kernel_spmd(
            nc, in_maps, core_ids=list(range(N_CORES)), trace=trace
        )
    except Exception:
        # the first PJRT compile in a fresh container can fail transiently;
        # one retry reuses the primed NEFF cache
        res = run_bass_kernel_spmd(
            nc, in_maps, core_ids=list(range(N_CORES)), trace=trace
        )
    full = np.concatenate([r["out"] for r in res.results], axis=0)  # [16, N]
    out = np.ascontiguousarray(full.T).astype(np.float32)  # [N, 16]
    return out, res


def kernel(**inputs):
    out, _ = kernel_with_results(trace=False, **inputs)
    return out


# revision 20
# speedup vs baseline: 1.7953x; 1.1071x over previous
"""Trainium2 Bass kernel for nn_EntityMapping (P=16 independent MLPs over a
shared entity batch).

Sharding: the 16 partition-MLPs are split across 8 NeuronCores (2 per core,
expert-parallel); the embedding batch is replicated. Activations are kept
feature-major [feature, batch] on-chip so every layer is a chain of
128x128-stationary matmuls with the batch streaming through the PE array.

All matmuls run in fp8 (e4m3) DoubleRow perf mode: one instruction contracts
over 256 rows (two 128-k-chunks) at 0.5 PE cycles per output column — 4x the
per-k-chunk throughput of fp32r. Inputs/weights are quantized host-side with
power-of-2 scales chosen so each layer's PSUM lands directly in the next
layer's fp8 units (SE*SW0 = SH1, SH1*SW1 = SH2): the relu is then a pure
(x + bias, max 0) with no rescale — a single activation on ACT or a single
dual-op tensor_scalar on DVE. GPSIMD cannot read PSUM, so relu work is split
ACT/DVE only; to amortize the per-instruction PSUM-access overhead, batch
chunks are processed in PAIRS: the same j-tile of two adjacent 512-column
chunks lands in adjacent PSUM banks and one [128, 2x512] relu (the bias is
per-feature, so it is shared) covers both. The final w2-dot runs as 2
DoubleRow matmuls per MLP/chunk into PSUM partitions 0/32 and one strided
sigmoid [2, 1024] covers both MLPs and both chunks.

Quantization error (validated on host vs the fp32 reference): rel_fro
~3.2e-3, max |pre-fp8 activation| ~104 of the 240 e4m3 range.
"""

import os
import sys

import numpy as np

if "jax" not in sys.modules and os.environ.get("JAX_PLATFORMS") == "cpu":
    # don't let a cpu pin hide the axon/neuron backend the kernel runs on
    os.environ["JAX_PLATFORMS"] = ""

try:
    import concourse.bass as bass  # noqa: F401
except ImportError:  # harness runs kernel.py from a bare directory
    sys.path.insert(0, "/opt/trn_rl_repo")

import ml_dtypes
import concourse.mybir as mybir
import concourse.tile as tile
from concourse import bacc
from concourse.bass_utils import run_bass_kernel_spmd

F32 = mybir.dt.float32
F8 = mybir.dt.float8e4
RELU = mybir.ActivationFunctionType.Relu
SIGMOID = mybir.ActivationFunctionType.Sigmoid
DR = mybir.MatmulPerfMode.DoubleRow
ADD = mybir.AluOpType.add
MAX = mybir.AluOpType.max

F8NP = ml_dtypes.float8_e4m3  # TRN-style e4m3 (max +-240), not OCP e4m3fn

P_TOTAL = 16  # independent MLP partitions
E = 512  # entity/embedding dim
H = 512  # hidden dim
N = 8192  # batch (entities)
N_CORES = 8
P_PER = P_TOTAL // N_CORES  # 2 MLPs per core
JC = 4  # 128-wide output-feature tiles per layer
KP = 2  # DoubleRow pairs per 512-deep contraction (2 x 256)
NCH = 512  # batch columns per chunk (= PSUM bank of fp32)
NDC = N // (2 * NCH)  # 8 double-chunks
NW = P_PER * JC * KP  # 16 DoubleRow weight tiles [128,2,128] per layer

# power-of-2 quantization scales; SE*SW0 = SH1 and SH1*SW1 = SH2 make the
# PSUM arrive already in the next layer's fp8 units (relu needs no rescale)
SE, SW0, SW1, SW2 = 2.0, 16.0, 2.0, 128.0
SH1 = SE * SW0  # 32
SH2 = SH1 * SW1  # 64
SC2 = 1.0 / (SH2 * SW2)  # 1/8192, applied inside the sigmoid activation

WARMUP_MM = 2  # junk matmuls to bridge the first DMA-semaphore sync


def _build():
    nc = bacc.Bacc(
        "TRN2", target_bir_lowering=False, debug=False, num_devices=N_CORES
    )
    # All inputs pre-quantized and pre-packed on host into SBUF layout:
    eT_dram = nc.dram_tensor("eT", [128, KP * 2 * N], F8, kind="ExternalInput")
    w0_dram = nc.dram_tensor("w0", [128, NW * 2 * 128], F8, kind="ExternalInput")
    w1_dram = nc.dram_tensor("w1", [128, NW * 2 * 128], F8, kind="ExternalInput")
    # dual-fp8 Ldweights needs a stationary free dim >= 32 and DoubleRow
    # matmuls need dst partition 0: w2 is padded to 64 zero columns with
    # MLP p's vector at column 32*p, so one accumulated PSUM region holds
    # p0's dot at partition 0 and p1's at partition 32
    w2_dram = nc.dram_tensor(
        "w2", [128, P_PER * KP * 2 * 64], F8, kind="ExternalInput"
    )
    b0_dram = nc.dram_tensor("b0", [128, P_PER * JC], F32, kind="ExternalInput")
    b1_dram = nc.dram_tensor("b1", [128, P_PER * JC], F32, kind="ExternalInput")
    b2_dram = nc.dram_tensor("b2", [P_PER, 1], F32, kind="ExternalInput")
    out_dram = nc.dram_tensor("out", [P_PER, N], F32, kind="ExternalOutput")

    eT_v = eT_dram.rearrange("p (kp two n) -> p kp two n", kp=KP, two=2)

    with tile.TileContext(nc) as tc:
        with (
            tc.tile_pool(name="wconst", bufs=1) as wconst,
            tc.tile_pool(name="warm", bufs=1) as warm_pool,
            tc.tile_pool(name="et", bufs=1) as et_pool,
            tc.tile_pool(name="h1", bufs=3) as h1_pool,
            tc.tile_pool(name="h2", bufs=3) as h2_pool,
            tc.tile_pool(name="osb", bufs=3) as out_pool,
            # one shared 4-tile PSUM rotation (2 banks each = all 8 banks):
            # L2 uses the same pool as the layer matmuls, so buffering goes
            # where the pipeline needs it instead of a dedicated L2 bank
            tc.tile_pool(name="mmps", bufs=4, space="PSUM") as ps_mm,
        ):
            # --- PE warmup: junk DoubleRow matmuls on a memset tile so the
            # PE clock is ramped when the first real matmul issues ---
            wm = warm_pool.tile([128, 2, 640], F8, tag="wm")
            nc.gpsimd.memset(wm[:], 0.0)
            ps_warm = ps_mm.tile([128, 2, NCH], F32, tag="mm")
            for i in range(WARMUP_MM):
                nc.tensor.matmul(
                    ps_warm[:, 0, :],
                    wm[:, :, 0:128],
                    wm[:, :, 128:640],
                    start=(i == 0),
                    stop=(i == WARMUP_MM - 1),
                    perf_mode=DR,
                )

            # --- whole eT resident in SBUF (32KB/partition in fp8); issued
            # on the ACT HW-DGE queue (ACT is idle this early), column-split
            # so chunk 0 unblocks immediately ---
            et = et_pool.tile([128, KP, 2, N], F8, tag="et")
            for lo, hi in ((0, 1024), (1024, 4096), (4096, N)):
                for q in range(KP):
                    for t in range(2):
                        nc.scalar.dma_start(
                            et[:, q, t, lo:hi], eT_v[:, q, t, lo:hi]
                        )

            # --- weights + consts on the SP HW-DGE queue, in consumption
            # order: w0 pieces (p,j), small consts, w1 pieces ---
            w0s = wconst.tile([128, NW, 2, 128], F8, tag="w0")
            w1s = wconst.tile([128, NW, 2, 128], F8, tag="w1")
            w2s = wconst.tile([128, P_PER, KP, 2, 64], F8, tag="w2")
            b0s = wconst.tile([128, P_PER * JC], F32, tag="b0")
            b1s = wconst.tile([128, P_PER * JC], F32, tag="b1")
            b2s = wconst.tile([33, 1], F32, tag="b2")

            w0sv = w0s[:].rearrange("p i two m -> p (i two m)")
            w1sv = w1s[:].rearrange("p i two m -> p (i two m)")
            PIECE = JC * KP * 2 * 128  # one per-MLP half-layer = 2KB fp8

            def w_dma(dst_flat, src, p):
                q0 = p * PIECE
                nc.sync.dma_start(
                    dst_flat[:, q0 : q0 + PIECE],
                    src[:, q0 : q0 + PIECE],
                )

            # coarse, consumption-ordered weight stream: 4 big pieces land
            # everything within ~4us so the first real matmuls never stall
            w_dma(w0sv, w0_dram, 0)
            w_dma(w0sv, w0_dram, 1)
            w_dma(w1sv, w1_dram, 0)
            w_dma(w1sv, w1_dram, 1)
            nc.sync.dma_start(b0s[:], b0_dram[:])
            nc.sync.dma_start(b1s[:], b1_dram[:])
            nc.sync.dma_start(
                w2s[:].rearrange("p a q two m -> p (a q two m)"), w2_dram[:]
            )
            nc.sync.dma_start(b2s[0:1, :], b2_dram[0:1, :])
            nc.sync.dma_start(b2s[32:33, :], b2_dram[1:2, :])

            # --- relu dispatch: alternate ACT/DVE (ACT also owns the
            # per-double-chunk sigmoid, DVE is slightly slower per op) ---
            relu_ctr = [0]

            def relu(dst, ps, bias):
                # flat [128, 1024] APs (both sides are contiguous)
                dst = dst.rearrange("p a b -> p (a b)")
                src = ps[:].rearrange("p a b -> p (a b)")
                relu_ctr[0] += 1
                if relu_ctr[0] % 2:
                    nc.vector.tensor_scalar(dst, src, bias, 0.0, ADD, MAX)
                else:
                    nc.scalar.activation(dst, src, RELU, bias=bias)

            # h tiles hold one double-chunk: [feature128, j, cc, col]
            def l0(p, dc):
                n0 = dc * 2 * NCH
                h1 = h1_pool.tile([128, JC, 2, NCH], F8, tag="h1")
                for j in range(JC):
                    ps = ps_mm.tile([128, 2, NCH], F32, tag="mm")
                    for cc in range(2):
                        nn = n0 + cc * NCH
                        for q in range(KP):
                            wi = (p * JC + j) * KP + q
                            nc.tensor.matmul(
                                ps[:, cc, :],
                                w0s[:, wi, :, :],
                                et[:, q, :, nn : nn + NCH],
                                start=(q == 0),
                                stop=(q == KP - 1),
                                perf_mode=DR,
                            )
                    col = p * JC + j
                    relu(h1[:, j, :, :], ps, b0s[:, col : col + 1])
                return h1

            def l1(p, h1):
                h2 = h2_pool.tile([128, JC, 2, NCH], F8, tag="h2")
                for j in range(JC):
                    ps = ps_mm.tile([128, 2, NCH], F32, tag="mm")
                    for cc in range(2):
                        for q in range(KP):
                            wi = (p * JC + j) * KP + q
                            nc.tensor.matmul(
                                ps[:, cc, :],
                                w1s[:, wi, :, :],
                                h1[:, 2 * q : 2 * q + 2, cc, :],
                                start=(q == 0),
                                stop=(q == KP - 1),
                                perf_mode=DR,
                            )
                    col = p * JC + j
                    relu(h2[:, j, :, :], ps, b1s[:, col : col + 1])
                return h2

            def l2(dc, h2_by_p):
                n0 = dc * 2 * NCH
                r = ps_mm.tile([128, 2, NCH], F32, tag="mm")
                for cc in range(2):
                    for p, h2 in enumerate(h2_by_p):
                        for q in range(KP):
                            nc.tensor.matmul(
                                r[0:64, cc, :],
                                w2s[:, p, q, :, :],
                                h2[:, 2 * q : 2 * q + 2, cc, :],
                                start=(p == 0 and q == 0),
                                stop=(p == P_PER - 1 and q == KP - 1),
                                perf_mode=DR,
                            )
                o = out_pool.tile([33, 2, NCH], F32, tag="o")
                nc.scalar.activation(
                    o[0:1, :, :].rearrange("p a b -> p (a b)"),
                    r[0:1, :, :].rearrange("p a b -> p (a b)"),
                    SIGMOID, bias=b2s[0:1, :], scale=SC2,
                )
                nc.scalar.activation(
                    o[32:33, :, :].rearrange("p a b -> p (a b)"),
                    r[32:33, :, :].rearrange("p a b -> p (a b)"),
                    SIGMOID, bias=b2s[32:33, :], scale=SC2,
                )
                nc.sync.dma_start(out_dram[0:1, n0 : n0 + 2 * NCH], o[0:1, :, :])
                nc.sync.dma_start(out_dram[1:2, n0 : n0 + 2 * NCH], o[32:33, :, :])

            # --- software-pipelined main loop over double-chunks: dc+1's L0
            # runs between dc's L1 and L2, giving every relu a full PE-block
            # of slack before its consumer ---
            h1s = [l0(0, 0), l0(1, 0)]
            for dc in range(NDC):
                h2s = [l1(0, h1s[0]), l1(1, h1s[1])]
                if dc + 1 < NDC:
                    h1s = [l0(0, dc + 1), l0(1, dc + 1)]
                l2(dc, h2s)

    nc.compile()
    return nc


_NC_CACHE = None


def _get_nc():
    global _NC_CACHE
    if _NC_CACHE is None:
        _NC_CACHE = _build()
    return _NC_CACHE


def _q8(x, scale):
    return (np.asarray(x, dtype=np.float32) * scale).astype(F8NP)


def _make_in_maps(e_embedding, W0, b0, W1, b1, W2, b2):
    e = np.asarray(e_embedding, dtype=np.float32)
    W0 = np.asarray(W0, dtype=np.float32)
    b0 = np.asarray(b0, dtype=np.float32)
    W1 = np.asarray(W1, dtype=np.float32)
    b1 = np.asarray(b1, dtype=np.float32)
    W2 = np.asarray(W2, dtype=np.float32)
    b2 = np.asarray(b2, dtype=np.float32)

    # eT [E, N] -> [ki, kp, two, n] fp8 (replicated to all cores)
    eTq = np.ascontiguousarray(
        _q8(e.T, SE).reshape(KP, 2, 128, N).transpose(2, 0, 1, 3).reshape(128, -1)
    )

    def wpack(W, s):  # [pp, 512, 512] -> [ki, (p j q two m)] fp8
        return np.ascontiguousarray(
            _q8(W, s)
            .reshape(P_PER, KP, 2, 128, JC, 128)
            .transpose(3, 0, 4, 1, 2, 5)
            .reshape(128, -1)
        )

    in_maps = []
    for cid in range(N_CORES):
        sl = slice(P_PER * cid, P_PER * (cid + 1))
        w0t = wpack(W0[sl], SW0)
        w1t = wpack(W1[sl], SW1)
        w2q = (
            _q8(W2[sl, :, 0], SW2)
            .reshape(P_PER, KP, 2, 128)
            .transpose(3, 0, 1, 2)
        )  # [128, p, q, two]
        w2t = np.zeros((128, P_PER, KP, 2, 64), dtype=F8NP)
        for p in range(P_PER):
            w2t[:, p, :, :, 32 * p] = w2q[:, p]
        w2t = np.ascontiguousarray(w2t.reshape(128, -1))
        b0t = np.ascontiguousarray(
            (b0[sl] * SH1).reshape(P_PER, JC, 128).transpose(2, 0, 1).reshape(128, -1)
        )
        b1t = np.ascontiguousarray(
            (b1[sl] * SH2).reshape(P_PER, JC, 128).transpose(2, 0, 1).reshape(128, -1)
        )
        b2t = np.ascontiguousarray(b2[sl].reshape(P_PER, 1))
        in_maps.append(
            {"eT": eTq, "w0": w0t, "b0": b0t, "w1": w1t, "b1": b1t,
             "w2": w2t, "b2": b2t}
        )
    return in_maps


def kernel_with_results(trace=False, **inputs):
    nc = _get_nc()
    in_maps = _make_in_maps(**inputs)
    try:
        res = run_bass_kernel_spmd(
            nc, in_maps, core_ids=list(range(N_CORES)), trace=trace
        )
    except Exception:
        # the first PJRT compile in a fresh container can fail transiently;
        # one retry reuses the primed NEFF cache
        res = run_bass_kernel_spmd(
            nc, in_maps, core_ids=list(range(N_CORES)), trace=trace
        )
    full = np.concatenate([r["out"] for r in res.results], axis=0)  # [16, N]
    out = np.ascontiguousarray(full.T).astype(np.float32)  # [N, 16]
    return out, res


def kernel(**inputs):
    out, _ = kernel_with_results(trace=False, **inputs)
    return out
